# revision 3
# baseline (speedup 1.0000x reference)
"""Trainium2 Bass kernel for the nGPT-style dense transformer block (v2).

Data-parallel: one batch element per NeuronCore.  Differences vs v1:
  * Wq/Wk/Wv/Wo column-normalized on HOST (f64) and shipped as fp8 (x16 —
    scale cancels through justnorm / the folded exp scale).
  * h^T shipped fp8; QKV + O projections run fp8 DoubleRow (K=256/matmul).
  * attention probabilities exp(logit-3) written as fp8; AV runs fp8
    DoubleRow over token-pair chunks; v is kept fp8.
  * scores stay bf16 K=64 band matmuls, emitted with alternating 64-row
    PE-array bands (tile pairing).
  * exp processed in [128,1024] tiles straight from PSUM.
  * h2 stays resident in SBUF f32 (no DRAM round-trip for the F2 residual).
  * weight-norm / reciprocal work moved off the hot engines; residual
    combines fused into scalar_tensor_tensor / tensor_tensor_reduce ops.
"""

import numpy as np
import ml_dtypes

import concourse.bass as bass
import concourse.mybir as mybir
import concourse.tile as tile
from concourse import bacc
BF16 = ml_dtypes.bfloat16
FP8 = ml_dtypes.float8_e4m3
F32 = mybir.dt.float32
BF = mybir.dt.bfloat16
F8 = mybir.dt.float8e4
MM8 = mybir.MatmulPerfMode.DoubleRow
AF = mybir.ActivationFunctionType
AX = mybir.AxisListType
ALU = mybir.AluOpType

P = 128
T = 1024
C = 1024
H = 16
D = 64
F = 8192
NCORES = 8
TCH = T // P   # 8 token chunks
CCH = C // P   # 8 channel chunks
KCH = (F // 2) // P  # 32 chunks of the 4096-dim MLP mid

BASE_SCALE = 0.03125
ATTN_ALPHA_INIT = 0.05
MLP_ALPHA_INIT = 0.05
SQK_INIT = 1.0
SUV_INIT = 1.0

WSCALE = 16.0     # host scale on normalized W columns (cancels exactly)
EXP_BIAS = -3.0   # exp(logit + bias); positive row scale cancels in justnorm
YSCALE = 1.0 / 16.0  # y -> fp8 eviction scale (cancels in justnorm)

_COMPILED: dict = {}


def _rsqrt(nc, x):
    """x <- 1/sqrt(x), elementwise on a small [128, n] tile."""
    nc.vector.reciprocal(x, x)
    nc.scalar.sqrt(x, x)


class _Pools:
    def __init__(self, tc):
        self.tc = tc
        self._open = {}

    def open(self, name, **kw):
        cm = self.tc.tile_pool(name=name, **kw)
        pool = cm.__enter__()
        self._open[name] = cm
        return pool

    def close(self, *names):
        for name in names:
            cm = self._open.pop(name)
            cm.__exit__(None, None, None)

    def close_all(self):
        for name in reversed(list(self._open)):
            self.close(name)


def _declare_io(nc):
    io = {}
    io["htf"] = nc.dram_tensor("htf", [P, CCH, T], F8, kind="ExternalInput")
    io["htm"] = nc.dram_tensor("htm", [TCH, P, C], F32, kind="ExternalInput")
    io["wq"] = nc.dram_tensor("wq", [P, CCH, C], F8, kind="ExternalInput")
    io["wk"] = nc.dram_tensor("wk", [P, CCH, C], F8, kind="ExternalInput")
    io["wv"] = nc.dram_tensor("wv", [P, CCH, C], F8, kind="ExternalInput")
    io["wo"] = nc.dram_tensor("wo", [P, CCH, C], F8, kind="ExternalInput")
    io["wfc"] = nc.dram_tensor("wfc", [16, P, CCH, 512], F8, kind="ExternalInput")
    io["wpj"] = nc.dram_tensor("wpj", [P, KCH, C], F8, kind="ExternalInput")
    io["esc8"] = nc.dram_tensor("esc8", [P, H], F32, kind="ExternalInput")
    io["ident"] = nc.dram_tensor("ident", [P, P], BF, kind="ExternalInput")
    io["out"] = nc.dram_tensor("out", [TCH, P, C], F32, kind="ExternalOutput")
    return io


def _emit(nc, tc, io, lr_a: float, lr_m: float, stop_after: str = "full"):
    def _dump_and_stop(pl, nc, out_d, srcs):
        """DMA up to 8 [P, C]-shaped f32 views of srcs to out and stop."""
        dmp = pl.open("dmp", bufs=2)
        for i, s in enumerate(srcs[:TCH]):
            dt_ = dmp.tile([P, C], F32, name=f"dt{i}", tag="dt")
            nc.vector.tensor_copy(dt_, s)
            nc.sync.dma_start(out=out_d.ap()[i], in_=dt_)
        pl.close("dmp")
        pl.close_all()
    htf_d, htm_d = io["htf"], io["htm"]
    wq_d, wk_d, wv_d, wo_d = io["wq"], io["wk"], io["wv"], io["wo"]
    wfc_d, wpj_d, esc8_d, ident_d = io["wfc"], io["wpj"], io["esc8"], io["ident"]
    out_d = io["out"]

    pl = _Pools(tc)

    # ---------------- long-lived constants ----------------
    consts = pl.open("consts", bufs=1)
    ident = consts.tile([P, P], BF)
    nc.sync.dma_start(out=ident, in_=ident_d.ap())
    esc8 = consts.tile([P, H], F32)
    nc.sync.dma_start(out=esc8, in_=esc8_d.ap())
    ebias = consts.tile([P, 1], F32)
    nc.vector.memset(ebias, EXP_BIAS)
    identf = consts.tile([P, P], F32)
    nc.vector.tensor_copy(identf, ident)
    small = pl.open("small", bufs=4)

    # h2 (f32, SBUF-resident through F2) -- opened early for stack discipline
    h2ap = pl.open("h2ap", bufs=1)
    h2a = h2ap.tile([P, TCH, C], F32)

    # ============ Phase P: QKV projections (fp8 DoubleRow) ============
    wop = pl.open("wop", bufs=1)
    vescp = pl.open("vescp", bufs=1)     # vf8, esc_all: die after Phase A
    qkp = pl.open("qkp", bufs=1)         # qhat, kbf: die after Phase T
    wqkv3 = pl.open("wqkv3", bufs=1)

    qhat = qkp.tile([P, TCH, C], BF)     # normalized q, TM
    kbf = qkp.tile([P, TCH, C], BF)      # raw k (bf16), TM
    # v in fp8, two half-zeroed copies: vz[s] has head-parity s features
    # live and the other parity zero, so AV DoubleRow can use M=128
    # stationaries that write both sub-heads' PSUM rows in one chain.
    vz = [vescp.tile([P, TCH, C], F8, name=f"vz{s}") for s in range(2)]
    esc_all = vescp.tile([P, TCH, H], F32)  # exp scale per (tk-chunk, head)
    for s in range(2):
        nc.vector.memset(vz[s], 0)

    htfp = pl.open("htfp", bufs=1)
    qscr = pl.open("qscr", bufs=2)
    tpps = pl.open("tp_psum", bufs=2, space="PSUM")
    qkvps = pl.open("qkv_psum", bufs=1, space="PSUM")
    htf = htfp.tile([P, CCH, T], F8)
    w_sbs = {
        "wq": wqkv3.tile([P, CCH, C], F8, name="wq_sb"),
        "wk": wqkv3.tile([P, CCH, C], F8, name="wk_sb"),
        "wv": wqkv3.tile([P, CCH, C], F8, name="wv_sb"),
        "wo": wop.tile([P, CCH, C], F8, name="wo_sb"),
    }
    for ci in range(CCH):
        nc.sync.dma_start(out=htf[:, ci, :], in_=htf_d.ap()[:, ci, :])
        for nm, wd in (("wq", wq_d), ("wk", wk_d), ("wv", wv_d)):
            nc.sync.dma_start(out=w_sbs[nm][:, ci, :], in_=wd.ap()[:, ci, :])
    for it in range(TCH):
        psq = qkvps.tile([P, 2, 512], F32, name="psq", tag="psq")
        psk = qkvps.tile([P, 2, 512], F32, name="psk", tag="psk")
        psv = qkvps.tile([P, 2, 512], F32, name="psv", tag="psv")
        for cp in range(CCH // 2):
            lhs = htf[:, 2 * cp:2 * cp + 2, it * P:(it + 1) * P]
            for ps, wnm in ((psq, "wq"), (psk, "wk"), (psv, "wv")):
                for hf in range(2):
                    nc.tensor.matmul(
                        ps[:, hf], lhs,
                        w_sbs[wnm][:, 2 * cp:2 * cp + 2, hf * 512:(hf + 1) * 512],
                        perf_mode=MM8,
                        start=(cp == 0), stop=(cp == CCH // 2 - 1),
                    )
        psqv = psq.rearrange("p a b -> p (a b)")
        pskv = psk.rearrange("p a b -> p (a b)")
        # ---- Q: per-head norms straight from PSUM ----
        sq = qscr.tile([P, C], F32, name="sq", tag="sq")
        nc.scalar.square(sq, psqv)
        rq = small.tile([P, H], F32, name="rq", tag="rq")
        nc.vector.reduce_sum(rq, sq.rearrange("p (h d) -> p h d", h=H), axis=AX.X)
        _rsqrt(nc, rq)
        nc.vector.tensor_mul(
            qhat[:, it, :].rearrange("p (h d) -> p h d", h=H),
            psq.rearrange("p a (g d) -> p (a g) d", d=D),
            rq.to_broadcast((P, H, D)),
        )
        # ---- K: norms -> exp scale; cast bf16 from PSUM ----
        sk = qscr.tile([P, C], F32, name="sk", tag="sq")
        nc.scalar.square(sk, pskv)
        rk = small.tile([P, H], F32, name="rk", tag="rk")
        nc.vector.reduce_sum(rk, sk.rearrange("p (h d) -> p h d", h=H), axis=AX.X)
        _rsqrt(nc, rk)
        nc.vector.tensor_mul(esc_all[:, it, :], rk, esc8)
        nc.scalar.copy(kbf[:, it, :], pskv)
        # ---- V: fp8 cast into the two half-zeroed copies (DVE, strided) ----
        # feature f = 512*a + 64*i + d  (a=hf half, i=0..7): head parity = i&1
        psv_v = psv.rearrange("p a (i d) -> p a i d", d=D)
        for s in range(2):
            nc.vector.tensor_copy(
                vz[s][:, it, :].rearrange("p (a i d) -> p a i d", a=2, d=D)
                [:, :, s::2, :],
                psv_v[:, :, s::2, :],
            )
    pl.close("qkv_psum", "qscr", "htfp", "wqkv3")
    if stop_after == "p":
        _dump_and_stop(pl, nc, out_d, [qhat[:, i, :] for i in range(TCH)])
        return

    # ============ Phase T: transpose qhat/k to feature-major ============
    yfmp = pl.open("yfmp", bufs=1, side="right")   # yfm fp8: dies after Phase O
    yfm = yfmp.tile([P, CCH, T], F8)
    htmp = pl.open("htmp", bufs=1, side="right")
    htm_all = htmp.tile([P, TCH, C], F32)
    rsa_all = htmp.tile([P, TCH], F32)
    hnscr = pl.open("hnscr", bufs=2, side="right")
    for it in range(TCH):
        nc.sync.dma_start(out=htm_all[:, it, :], in_=htm_d.ap()[it])
        nscr = hnscr.tile([P, C], F32, name="nscr", tag="nscr")
        nc.scalar.activation(nscr, htm_all[:, it, :], AF.Square,
                             accum_out=rsa_all[:, it:it + 1])
    _rsqrt(nc, rsa_all)
    nc.vector.tensor_scalar_mul(rsa_all, rsa_all, 1.0 - lr_a)
    pl.close("hnscr")
    qkfmp = pl.open("qkfmp", bufs=1, side="right")  # qfm, kfm: die after Phase A
    qfm = qkfmp.tile([P, CCH, T], BF)
    kfm = qkfmp.tile([P, CCH, T], BF)
    for ci in range(CCH):
        for src, dst in ((qhat, qfm), (kbf, kfm)):
            for g in range(2):
                tp = tpps.tile([P, 4, P], BF, name="tp", tag="tp")
                for jj in range(4):
                    it = g * 4 + jj
                    nc.tensor.transpose(
                        tp[:, jj], src[:, it, ci * P:(ci + 1) * P], ident
                    )
                nc.vector.tensor_copy(
                    dst[:, ci, g * 512:(g + 1) * 512],
                    tp.rearrange("p a b -> p (a b)"),
                )
    for ci in range(CCH):
        nc.sync.dma_start(out=w_sbs["wo"][:, ci, :], in_=wo_d.ap()[:, ci, :])
    pl.close("tp_psum", "qkp")
    if stop_after == "t":
        _dump_and_stop(pl, nc, out_d, [qfm[:, i, :] for i in range(TCH)])
        return

    # ============ Phase A: attention ============
    # scores bf16 on alternating 64-row PE bands; exp -> fp8 p; AV fp8-DR.
    aps = pl.open("att_psum", bufs=1, space="PSUM")
    app = pl.open("att_p", bufs=2)
    for hp in range(H // 2):
        ypsum = aps.tile([P, 2, 512], F32, name="ypsum", tag="ypsum", bufs=2)
        p_sb = [
            app.tile([P, TCH, T], F8, name=f"p{sub}", tag=f"p{sub}")
            for sub in range(2)
        ]
        for tk in range(TCH):
            sps = []
            for sub in range(2):
                prow = sub * D
                sp = aps.tile([P, 2, 512], F32, name="sp", tag="sp", bufs=2)
                for hf in range(2):
                    nc.tensor.matmul(
                        sp[:, hf],
                        kfm[prow:prow + D, hp, tk * P:(tk + 1) * P],
                        qfm[prow:prow + D, hp, hf * 512:(hf + 1) * 512],
                        start=True, stop=True,
                    )
                sps.append(sp)
            for sub in range(2):
                h = hp * 2 + sub
                nc.scalar.activation(
                    out=p_sb[sub][:, tk, :],
                    in_=sps[sub].rearrange("p a b -> p (a b)"),
                    func=AF.Exp,
                    scale=esc_all[:, tk, h:h + 1],
                    bias=ebias,
                )
            if tk % 2 == 1:
                m = tk // 2
                for sub in range(2):
                    for hf in range(2):
                        nc.tensor.matmul(
                            ypsum[:, hf],
                            vz[sub][:, 2 * m:2 * m + 2, hp * P:(hp + 1) * P],
                            p_sb[sub][:, 2 * m:2 * m + 2,
                                      hf * 512:(hf + 1) * 512],
                            perf_mode=MM8,
                            start=(m == 0 and sub == 0),
                            stop=(m == TCH // 2 - 1 and sub == 1),
                        )
        nc.vector.tensor_scalar_mul(
            yfm[:, hp, :], ypsum.rearrange("p a b -> p (a b)"), YSCALE
        )
    pl.close("att_psum", "att_p", "qkfmp", "vescp")
    if stop_after == "a":
        _dump_and_stop(pl, nc, out_d, [yfm[:, i, :] for i in range(TCH)])
        return

    # ============ Phase O: output projection (fp8-DR) + attn residual ============
    opsp = pl.open("o_psum", bufs=3, space="PSUM")
    oscr = pl.open("o_scr", bufs=3)
    for it in range(TCH):
        ops = opsp.tile([P, 2, 512], F32, name="ops", tag="ops")
        for cp in range(CCH // 2):
            lhs = yfm[:, 2 * cp:2 * cp + 2, it * P:(it + 1) * P]
            for hf in range(2):
                nc.tensor.matmul(
                    ops[:, hf], lhs,
                    w_sbs["wo"][:, 2 * cp:2 * cp + 2, hf * 512:(hf + 1) * 512],
                    perf_mode=MM8,
                    start=(cp == 0), stop=(cp == CCH // 2 - 1),
                )
        opsv = ops.rearrange("p a b -> p (a b)")
        sb = small.tile([P, 1], F32, name="sb", tag="sb")
        t1 = oscr.tile([P, C], F32, name="t1", tag="t1")
        scr0 = oscr.tile([P, C], F32, name="scr0", tag="scr0")
        acc = oscr.tile([P, C], F32, name="acc", tag="acc")
        # sb = sum(h_att^2); scr0 is scratch (single PSUM read: ACT square)
        nc.scalar.activation(scr0, opsv, AF.Square, accum_out=sb)
        _rsqrt(nc, sb)
        nc.vector.tensor_scalar_mul(sb, sb, lr_a)
        nc.scalar.mul(t1, htm_all[:, it, :], rsa_all[:, it:it + 1])
        nc.vector.tensor_scalar_mul(acc, opsv, sb)  # lr*justnorm(h_att)
        nc.vector.tensor_add(acc, acc, t1)
        s2 = small.tile([P, 1], F32, name="s2", tag="s2")
        nc.scalar.activation(t1, acc, AF.Square, accum_out=s2)
        _rsqrt(nc, s2)
        nc.scalar.mul(h2a[:, it, :], acc, s2)     # h2 = justnorm(...)
    pl.close("o_psum", "o_scr", "htmp", "yfmp", "wop")
    if stop_after == "o":
        _dump_and_stop(pl, nc, out_d, [h2a[:, i, :] for i in range(TCH)])
        return

    # ============ Phase T2: transpose h2 (f32) to feature-major fp8 ============
    h2fmp = pl.open("h2fmp", bufs=1, side="right")  # h2fm: dies after F1
    h2fm = h2fmp.tile([P, CCH, T], F8)   # scaled x8 into fp8 range
    tpps2 = pl.open("tp2_psum", bufs=3, space="PSUM")
    for ci in range(CCH):
        for g in range(2):
            tp2 = tpps2.tile([P, 4, P], F32, name="tp2", tag="tp2")
            for jj in range(4):
                it = g * 4 + jj
                nc.tensor.transpose(
                    tp2[:, jj], h2a[:, it, ci * P:(ci + 1) * P], identf
                )
            nc.vector.tensor_scalar_mul(
                h2fm[:, ci, g * 512:(g + 1) * 512],
                tp2.rearrange("p a b -> p (a b)"), 8.0,
            )
    pl.close("tp2_psum")

    # ============ Phase F1: MLP up + SwiGLU (feature-major out) ============
    xmp = pl.open("xmp", bufs=1)
    xm = xmp.tile([P, KCH, T], F8)  # x_mlp feature-major (scaled; justnorm cancels)
    wpjp = pl.open("wpjp", bufs=1)
    wpj = wpjp.tile([P, KCH, C], F8)
    for q in range(4):
        nc.sync.dma_start(
            out=wpj[:, q * 8:(q + 1) * 8, :], in_=wpj_d.ap()[:, q * 8:(q + 1) * 8, :]
        )
    f1w = pl.open("f1w", bufs=3, side="right")
    f1ps = pl.open("f1_psum", bufs=2, space="PSUM")
    f1scr = pl.open("f1scr", bufs=2, side="right")
    for j in range(8):
        wu = f1w.tile([P, CCH, 512], F8, name="wu", tag="wu")
        nc.sync.dma_start(out=wu, in_=wfc_d.ap()[j])
        wvt = f1w.tile([P, CCH, 512], F8, name="wvt", tag="wvt")
        nc.sync.dma_start(out=wvt, in_=wfc_d.ap()[j + 8])
        for so in range(4):
            oc = j * 4 + so
            m0 = so * P
            up = f1ps.tile([P, 2, 512], F32, name="up", tag="up")
            vp = f1ps.tile([P, 2, 512], F32, name="vp", tag="vp")
            for cp in range(CCH // 2):
                for hf in range(2):
                    nc.tensor.matmul(
                        up[:, hf], wu[:, 2 * cp:2 * cp + 2, m0:m0 + P],
                        h2fm[:, 2 * cp:2 * cp + 2, hf * 512:(hf + 1) * 512],
                        perf_mode=MM8,
                        start=(cp == 0), stop=(cp == CCH // 2 - 1),
                    )
            for cp in range(CCH // 2):
                for hf in range(2):
                    nc.tensor.matmul(
                        vp[:, hf], wvt[:, 2 * cp:2 * cp + 2, m0:m0 + P],
                        h2fm[:, 2 * cp:2 * cp + 2, hf * 512:(hf + 1) * 512],
                        perf_mode=MM8,
                        start=(cp == 0), stop=(cp == CCH // 2 - 1),
                    )
            sil = f1scr.tile([P, T], BF, name="sil", tag="sil")
            nc.scalar.activation(
                out=sil, in_=vp.rearrange("p a b -> p (a b)"), func=AF.Silu,
                scale=1.0 / 8.0,
            )
            nc.vector.tensor_mul(
                xm[:, oc, :], up.rearrange("p a b -> p (a b)"), sil
            )
    pl.close("f1scr", "f1w", "h2fmp", "f1_psum")

    # ============ Phase F2: MLP down (fp8-DR) + MLP residual ============
    f2ps = pl.open("f2_psum", bufs=3, space="PSUM")
    f2scr = pl.open("f2scr", bufs=3)
    for it in range(TCH):
        mp = f2ps.tile([P, 2, 512], F32, name="mp", tag="mp")
        for kp in range(KCH // 2):
            for hf in range(2):
                nc.tensor.matmul(
                    mp[:, hf], xm[:, 2 * kp:2 * kp + 2, it * P:(it + 1) * P],
                    wpj[:, 2 * kp:2 * kp + 2, hf * 512:(hf + 1) * 512],
                    perf_mode=MM8,
                    start=(kp == 0), stop=(kp == KCH // 2 - 1),
                )
        mpv = mp.rearrange("p a b -> p (a b)")
        sb2 = small.tile([P, 1], F32, name="sb2", tag="sb2")
        t1 = f2scr.tile([P, C], F32, name="t1b", tag="t1b")
        scr2 = f2scr.tile([P, C], F32, name="scr2", tag="scr2")
        acc = f2scr.tile([P, C], F32, name="accb", tag="accb")
        nc.scalar.activation(scr2, mpv, AF.Square, accum_out=sb2)
        _rsqrt(nc, sb2)
        nc.vector.tensor_scalar_mul(sb2, sb2, lr_m)
        nc.scalar.mul(t1, h2a[:, it, :], 1.0 - lr_m)
        nc.vector.tensor_scalar_mul(acc, mpv, sb2)
        nc.vector.tensor_add(acc, acc, t1)
        s3 = small.tile([P, 1], F32, name="s3", tag="s3")
        nc.scalar.activation(t1, acc, AF.Square, accum_out=s3)
        _rsqrt(nc, s3)
        outt = f2scr.tile([P, C], F32, name="outt", tag="outt")
        nc.scalar.mul(outt, acc, s3)
        nc.sync.dma_start(out=out_d.ap()[it], in_=outt)

    pl.close_all()


def build_program(lr_a: float, lr_m: float, reps: int = 1, loop: int = 0,
                  stop_after: str = "full"):
    key = (round(lr_a, 12), round(lr_m, 12), reps, loop, stop_after)
    if key in _COMPILED:
        return _COMPILED[key]
    nc = bacc.Bacc("TRN2", target_bir_lowering=False, debug=False, num_devices=NCORES)
    with tile.TileContext(nc) as tc:
        io = _declare_io(nc)
        if loop:
            with tc.For_i(0, loop, 1):
                _emit(nc, tc, io, lr_a, lr_m, stop_after)
        else:
            for _ in range(reps):
                _emit(nc, tc, io, lr_a, lr_m, stop_after)
    nc.compile()
    _COMPILED[key] = nc
    return nc


def prep_inputs(h, Wq, Wk, Wv, Wo, Wfc, Wproj, sqk, suv, attn_alpha, mlp_alpha):
    """Host-side sharding/layout. Returns (in_maps list per core, lr_a, lr_m)."""
    h = np.asarray(h, np.float32)
    Wq, Wk, Wv, Wo = (np.asarray(w, np.float64) for w in (Wq, Wk, Wv, Wo))
    Wfc = np.asarray(Wfc, np.float32)
    Wproj = np.asarray(Wproj, np.float32)
    sqk = np.asarray(sqk, np.float64)
    suv = np.asarray(suv, np.float64)
    attn_alpha = np.asarray(attn_alpha, np.float64)
    mlp_alpha = np.asarray(mlp_alpha, np.float64)

    sqk_s = sqk * (SQK_INIT / BASE_SCALE)
    s_h = sqk_s.reshape(H, D)
    assert np.allclose(s_h, s_h[:, :1]), "sqk must be constant per head"
    s2 = (s_h[:, 0] ** 2) * np.sqrt(D)
    esc8 = np.ascontiguousarray(np.broadcast_to(s2.astype(np.float32), (P, H)))

    lr_a_v = np.abs(attn_alpha * (ATTN_ALPHA_INIT / BASE_SCALE))
    lr_m_v = np.abs(mlp_alpha * (MLP_ALPHA_INIT / BASE_SCALE))
    assert np.allclose(lr_a_v, lr_a_v[0]) and np.allclose(lr_m_v, lr_m_v[0]), \
        "alpha must be constant"
    lr_a = float(lr_a_v[0])
    lr_m = float(lr_m_v[0])

    def wt_tiles_norm_f8(W):  # [out, in] -> [128, CCH, out] fp8, unit cols x16
        Wn = W / np.linalg.norm(W, axis=0, keepdims=True)
        Wn = Wn * WSCALE
        return np.ascontiguousarray(
            Wn.T.reshape(CCH, P, W.shape[0]).transpose(1, 0, 2)
        ).astype(FP8)

    wq_t, wk_t, wv_t, wo_t = (wt_tiles_norm_f8(w) for w in (Wq, Wk, Wv, Wo))

    suv_s = suv * (SUV_INIT / 1.0 * np.sqrt(C))
    wfc_f = (Wfc.astype(np.float64) * suv_s[:, None]).astype(np.float32)  # [F, C]
    wfc_t = np.ascontiguousarray(
        np.clip(wfc_f.T.reshape(CCH, P, 16, 512).transpose(2, 1, 0, 3), -224, 224)
    ).astype(FP8)
    wpj_t = np.ascontiguousarray(
        np.clip(Wproj.T.reshape(KCH, P, C).transpose(1, 0, 2) * 64.0, -224, 224)
    ).astype(FP8)

    shared = {
        "wq": wq_t, "wk": wk_t, "wv": wv_t, "wo": wo_t,
        "wfc": wfc_t, "wpj": wpj_t, "esc8": esc8,
        "ident": np.eye(P, dtype=np.float32).astype(BF16),
    }
    in_maps = []
    for b in range(NCORES):
        htf = np.ascontiguousarray(
            h[b].T.reshape(CCH, P, T).transpose(1, 0, 2)
        ).astype(FP8)
        htm = np.ascontiguousarray(h[b].reshape(TCH, P, C))
        in_maps.append({"htf": htf, "htm": htm, **shared})
    return in_maps, lr_a, lr_m


def kernel(h, Wq, Wk, Wv, Wo, Wfc, Wproj, sqk, suv, attn_alpha, mlp_alpha):
    in_maps, lr_a, lr_m = prep_inputs(
        h, Wq, Wk, Wv, Wo, Wfc, Wproj, sqk, suv, attn_alpha, mlp_alpha
    )
    nc = build_program(lr_a, lr_m)
    from concourse.bass_utils import run_bass_kernel_spmd

    res = run_bass_kernel_spmd(nc, in_maps, core_ids=list(range(NCORES)))
    out = np.stack(
        [res.results[b]["out"].reshape(T, C) for b in range(NCORES)], axis=0
    )
    return out.astype(np.float32)
